# revision 16
# baseline (speedup 1.0000x reference)
"""Ball-query kernel for Trainium2 (Bass/Tile), 8 NeuronCores.

Problem: for each batch b (8 total) and each query point m (4096), return the
first 32 source indices n (in increasing n) with ||q_m - p_n||^2 < 0.2^2,
padding unused slots with the first valid index. Queries == sources (xyz).

Sharding: data-parallel over batch, one batch per core (8 cores).

Per-core algorithm (N=4096 queries x 4096 sources):
  - PE computes dot[m, n] = q_m . p_n per 128-query block (K=3 matmul).
  - DVE scalar_tensor_tensor: s = 2*dot - plus, where plus[m,n] = sq[m]+sq[n]
    (s == -d2 with bit-exact match to the reference's rounding order).
  - DVE STT: keys = (s > -r^2) * (4096 - n)  -> valid keys descending encode
    ascending indices; invalid -> 0.
  - 4 rounds of vector.max (top-8, descending) + match_replace to extract the
    32 largest keys = first 32 valid indices, in order.
  - Pad empty slots (key 0) with the first valid key; idx = 4096 - key.
"""

import numpy as np

N = 4096
NS = 32
R2 = 0.04
NCORES = 8
BLK = 128
NBLK = N // BLK   # 32
CH = 2048         # psum chunk (4 banks)
NCH = N // CH     # 2
MM = 512          # matmul free-dim per instruction (1 bank)


def _build_bass():
    import concourse.bass as bass
    import concourse.mybir as mybir
    from concourse import bacc, tile

    Alu = mybir.AluOpType
    f32 = mybir.dt.float32

    nc = bacc.Bacc(
        "TRN2", target_bir_lowering=False, debug=False, num_devices=NCORES
    )

    xyzT_d = nc.dram_tensor("xyzT", [3, N], f32, kind="ExternalInput")
    # sqA = [sqrep | sqq | inegrep]: per-row [sq(n) x N, sq_q blocks x 32, 4096-n x N]
    sqA_d = nc.dram_tensor("sqA", [128, 2 * N + NBLK], f32, kind="ExternalInput")
    out_d = nc.dram_tensor("out", [N, NS], mybir.dt.int32, kind="ExternalOutput")

    with tile.TileContext(nc) as tc:
        with (
            tc.tile_pool(name="const", bufs=1) as cpool,
            tc.tile_pool(name="psum", bufs=8, space="PSUM") as ppool,
            tc.tile_pool(name="work", bufs=2) as wpool,
            tc.tile_pool(name="small", bufs=3) as spool,
        ):
            xyzT_sb = cpool.tile([3, N], f32, tag="xyzT", name="xyzT_sb")
            nc.gpsimd.dma_start(xyzT_sb[:], xyzT_d.ap())
            sqA_sb = cpool.tile([128, 2 * N + NBLK], f32, tag="sqA", name="sqA_sb")
            nc.gpsimd.dma_start(sqA_sb[:], sqA_d.ap())
            def sqrep_sl(lo, hi):
                return sqA_sb[:, lo:hi]

            def sqq_sl(b):
                return sqA_sb[:, N + b : N + b + 1]

            def ineg_sl(lo, hi):
                return sqA_sb[:, N + NBLK + lo : N + NBLK + hi]

            for b in range(NBLK):
                # plus[m, n] = sq_q[m] + sq_src[n]
                plus = wpool.tile([128, N], f32, tag="plus", name="plus")
                for c in range(NCH):
                    nc.vector.tensor_scalar(
                        plus[:, c * CH : (c + 1) * CH],
                        sqrep_sl(c * CH, (c + 1) * CH),
                        sqq_sl(b),
                        None,
                        Alu.add,
                    )

                keys = wpool.tile([128, N], f32, tag="keys", name="keys")
                keys2 = wpool.tile([128, N], f32, tag="keys2", name="keys2")

                for j in range(N // MM):
                    ps = ppool.tile([128, MM], f32, tag="ps", name="ps")
                    nc.tensor.matmul(
                        ps[:],
                        xyzT_sb[:, b * BLK : (b + 1) * BLK],
                        xyzT_sb[:, j * MM : (j + 1) * MM],
                        start=True,
                        stop=True,
                    )
                    # s = 2*dot - plus  (== -d2, exact)
                    nc.vector.scalar_tensor_tensor(
                        keys2[:, j * MM : (j + 1) * MM],
                        ps[:],
                        2.0,
                        plus[:, j * MM : (j + 1) * MM],
                        Alu.mult,
                        Alu.subtract,
                    )
                for c in range(NCH):
                    # keys = (s > -r2) * (4096 - n)
                    nc.vector.scalar_tensor_tensor(
                        keys[:, c * CH : (c + 1) * CH],
                        keys2[:, c * CH : (c + 1) * CH],
                        -R2,
                        ineg_sl(c * CH, (c + 1) * CH),
                        Alu.is_gt,
                        Alu.mult,
                    )

                v8 = spool.tile([128, NS], f32, tag="v8", name="v8")
                nc.vector.max(v8[:, 0:8], keys[:])
                nc.vector.match_replace(keys2[:], v8[:, 0:8], keys[:], 0.0)
                nc.vector.max(v8[:, 8:16], keys2[:])
                nc.vector.match_replace(keys[:], v8[:, 8:16], keys2[:], 0.0)
                nc.vector.max(v8[:, 16:24], keys[:])
                nc.vector.match_replace(keys2[:], v8[:, 16:24], keys[:], 0.0)
                nc.vector.max(v8[:, 24:32], keys2[:])

                # pad empty slots (0) with first valid key, then idx = 4096 - key
                f8 = spool.tile([128, NS], f32, tag="f8", name="f8")
                nc.vector.tensor_scalar(f8[:], v8[:], 0.0, None, Alu.is_equal)
                t2 = spool.tile([128, NS], f32, tag="t2", name="t2")
                nc.vector.scalar_tensor_tensor(
                    t2[:], f8[:], v8[:, 0:1], v8[:], Alu.mult, Alu.add
                )
                idx = spool.tile([128, NS], mybir.dt.int32, tag="idx", name="idx")
                nc.vector.tensor_scalar(
                    idx[:], t2[:], -1.0, float(N), Alu.mult, Alu.add
                )
                nc.sync.dma_start(
                    out_d.ap()[b * BLK : (b + 1) * BLK, :], idx[:]
                )

    nc.compile()
    return nc


def kernel(xyz, xyz_new=None):
    from concourse.bass_utils import run_bass_kernel_spmd

    xyz = np.asarray(xyz, dtype=np.float32)
    nc = _build_bass()

    iota_neg = (np.float32(N) - np.arange(N, dtype=np.float32)).astype(np.float32)
    in_maps = []
    for b in range(NCORES):
        P = xyz[b]  # [4096, 3]
        x, y, z = P[:, 0], P[:, 1], P[:, 2]
        sq = (x * x + y * y) + z * z  # fp32, reference order
        row = np.concatenate([sq, np.zeros(NBLK, np.float32), iota_neg])
        sqA = np.broadcast_to(row, (128, 2 * N + NBLK)).copy()
        sqA[:, N : N + NBLK] = sq.reshape(NBLK, 128).T
        in_maps.append(
            {
                "xyzT": np.ascontiguousarray(P.T),
                "sqA": sqA,
            }
        )

    import os

    trace = bool(int(os.environ.get("BQ_TRACE", "0")))
    try:
        res = run_bass_kernel_spmd(
            nc, in_maps, core_ids=list(range(NCORES)), trace=trace
        )
    except ModuleNotFoundError:
        res = run_bass_kernel_spmd(nc, in_maps, core_ids=list(range(NCORES)))
    if trace and res.exec_time_ns is not None:
        print(f"HW exec time: {res.exec_time_ns} ns")
    return np.stack([res.results[b]["out"] for b in range(NCORES)]).astype(np.int32)


if __name__ == "__main__":
    rng = np.random.default_rng(0)
    xyz = rng.random((8, N, 3), dtype=np.float32)
    out = kernel(xyz)
    print(out.shape, out.dtype)
